# revision 19
# baseline (speedup 1.0000x reference)
"""Bahdanau decoder single step (batch=1) on 8 trn2 NeuronCores.

Sharding:
  - attention: encoder sequence S=16384 split 8x2048; softmax stats + context
    partials combined with one AllReduce([1032]).
  - LSTM gates: W_ih/W_hh rows interleaved so core k owns hidden slice
    [k*128,(k+1)*128) of each of the i,f,g,o gates; gate GEMVs run as
    PE weight-load GEMVs (exact fp32) with the ctx-column block deferred
    until after the AllReduce; h_new/c_new computed per-slice; h_new
    AllGathered for the classifier.
  - classifier: W_cls rows split 8x4000, computed per-core, host concats.

The x = tanh(q + E @ W_fc_enc^T) GEMM optionally runs in float32r
(hardware fast-fp32, ~2e-4 rel err) -- everything else is exact fp32.
"""

import os
import sys

if "/opt/trn_rl_repo" not in sys.path:
    sys.path.insert(0, "/opt/trn_rl_repo")

import numpy as np

H = 1024
O = 32000
S = 16384
NCORES = 8
SC = S // NCORES          # 2048 seq per core
HG = H // NCORES          # 128 hidden per core
CLS = O // NCORES         # 4000 vocab rows per core
KEMB = O // 128           # 250 contraction chunks for the embedded part
CLS_M = 125               # classifier output chunk (125 * 32 = 4000)
CLS_NM = CLS // CLS_M     # 32

# "f32r" (fast, ~2e-4 attention err) or "f32" (exact, slower PE)
XGEMM_DT = os.environ.get("BAHDANAU_XGEMM_DT", "f32r")
DEBUG_OUTS = os.environ.get("BAHDANAU_DEBUG", "0") == "1"

_CACHE = {}


def _legalize_single_wait(nc):
    """This walrus build accepts at most ONE sync-wait per instruction.
    Split any instruction with k>1 waits into (k-1) same-engine NoOps,
    each carrying one wait, placed immediately before it."""
    import bass_rust
    import concourse.mybir as mybir

    ctr = 0
    for f in nc.m.functions:
        for bb in f.blocks:
            insts = list(bb.instructions)
            if not any(i.sync_info and len(i.sync_info.on_wait) > 1 for i in insts):
                continue
            new_insts = []
            for inst in insts:
                si = inst.sync_info
                if si is not None and len(si.on_wait) > 1:
                    waits = list(si.on_wait)
                    for w in waits[:-1]:
                        ctr += 1
                        nop = mybir.InstNoOp(
                            name=f"waitsplit-{ctr}", engine=inst.engine
                        )
                        nop.sync_info = bass_rust.SyncInfo(
                            on_wait=[w], on_update=[]
                        )
                        new_insts.append(nop)
                    inst.sync_info = bass_rust.SyncInfo(
                        on_wait=[waits[-1]], on_update=list(si.on_update)
                    )
                new_insts.append(inst)
            bb.instructions = new_insts
    return ctr


def _build():
    import concourse.bass as bass
    import concourse.mybir as mybir
    from concourse.tile import TileContext

    f32 = mybir.dt.float32
    xdt = mybir.dt.float32r if XGEMM_DT == "f32r" else f32
    AF = mybir.ActivationFunctionType

    nc = bass.Bass()

    # ---- per-core inputs (host pre-sliced / pre-transposed)
    eT = nc.dram_tensor("eT", [H, SC], xdt, kind="ExternalInput")       # E^T slice
    e_ = nc.dram_tensor("e", [SC, H], f32, kind="ExternalInput")        # E slice
    wfce = nc.dram_tensor("wfce", [H, H], xdt, kind="ExternalInput")    # W_fc_encoder.T
    wfch = nc.dram_tensor("wfch", [H, H], f32, kind="ExternalInput")    # W_fc_hidden.T
    wat = nc.dram_tensor("wat", [128, 8], f32, kind="ExternalInput")    # attn w tiled
    h0t = nc.dram_tensor("h0t", [128, 8], f32, kind="ExternalInput")    # h0 tiled
    c0c = nc.dram_tensor("c0c", [128, 1], f32, kind="ExternalInput")    # c0 slice
    embt = nc.dram_tensor("embt", [128, KEMB], f32, kind="ExternalInput")
    wemb = nc.dram_tensor("wemb", [O, 512], f32, kind="ExternalInput")  # Wih[:, :O][rows].T
    wctx = nc.dram_tensor("wctx", [H, 512], f32, kind="ExternalInput")  # Wih[:, O:][rows].T
    whh = nc.dram_tensor("whh", [H, 512], f32, kind="ExternalInput")    # Whh[rows].T
    biasg = nc.dram_tensor("biasg", [128, 4], f32, kind="ExternalInput")
    wcls = nc.dram_tensor("wcls", [H, CLS], f32, kind="ExternalInput")  # Wcls[rows].T
    bcls = nc.dram_tensor("bcls", [CLS_M, CLS_NM], f32, kind="ExternalInput")

    # ---- per-core outputs
    attn_o = nc.dram_tensor("attn_o", [SC], f32, kind="ExternalOutput")  # [128,16] row-major
    c_o = nc.dram_tensor("c_o", [HG], f32, kind="ExternalOutput")
    h_o = nc.dram_tensor("h_o", [HG], f32, kind="ExternalOutput")
    cls_o = nc.dram_tensor("cls_o", [CLS], f32, kind="ExternalOutput")   # [125,32] row-major

    with TileContext(nc) as tc:
        with (
            tc.tile_pool(name="cst", bufs=1) as cst,
            tc.tile_pool(name="peT", bufs=1) as peT,
            tc.tile_pool(name="pfce", bufs=1) as pfce,
            tc.tile_pool(name="pfch", bufs=2) as pfch,
            tc.tile_pool(name="pe", bufs=3) as pe_pool,
            tc.tile_pool(name="pxT", bufs=3) as pxT,
            tc.tile_pool(name="pwg", bufs=8) as pwg,
            tc.tile_pool(name="pwcls", bufs=3) as pwcls,
            tc.tile_pool(name="psA", bufs=2, space="PSUM") as psA,
            tc.tile_pool(name="psS", bufs=3, space="PSUM") as psS,
            tc.tile_pool(name="psB", bufs=1, space="PSUM") as psB,
            tc.tile_pool(name="dram", bufs=1, space="DRAM") as dram,
        ):
            # ================= constants =================
            wat_sb = cst.tile([128, 8], f32)
            h0_sb = cst.tile([128, 8], f32)
            c0_sb = cst.tile([128, 1], f32)
            biasg_sb = cst.tile([128, 4], f32)
            emb_sb = cst.tile([128, KEMB], f32)
            bcls_sb = cst.tile([CLS_M, CLS_NM], f32)
            ones_col = cst.tile([128, 1], f32)
            ones_row = cst.tile([1, 128], f32)
            nc.sync.dma_start(wat_sb[:], wat[:])
            nc.sync.dma_start(h0_sb[:], h0t[:])
            nc.sync.dma_start(c0_sb[:], c0c[:])
            nc.sync.dma_start(biasg_sb[:], biasg[:])
            nc.sync.dma_start(emb_sb[:], embt[:])
            nc.sync.dma_start(bcls_sb[:], bcls[:])
            nc.vector.memset(ones_col[:], 1.0)
            nc.vector.memset(ones_row[:], 1.0)

            # ================= q = W_fc_hidden @ h0 =================
            # NOTE: matmul start=True clears has_written for the WHOLE psum
            # bank, so interleaved accumulation groups in one bank are
            # broken.  All GEMVs below therefore do single-shot matmuls
            # (start=stop=True) into a scratch psum tile and accumulate in
            # SBUF with DVE adds.
            q_sb = cst.tile([128, 8], f32)
            nc.vector.memset(q_sb[:], 0.0)
            for c in range(8):
                wfch_sb = pfch.tile([128, H], f32, name="wfch_sb")
                nc.sync.dma_start(wfch_sb[:], wfch[c * 128:(c + 1) * 128, :])
                scr_q = psS.tile([128, 8], f32, name="scr_q", tag="scr")
                for m in range(8):
                    nc.tensor.matmul(
                        scr_q[:, m:m + 1],
                        wfch_sb[:, m * 128:(m + 1) * 128],
                        h0_sb[:, c:c + 1],
                        start=True,
                        stop=True,
                    )
                nc.vector.tensor_add(q_sb[:], q_sb[:], scr_q[:])

            # ============ xT = tanh(q + W_fce^T.T @ E^T), scores ============
            eT_tiles = []
            for c in range(8):
                t = peT.tile([128, SC], xdt, name=f"eT{c}")
                nc.sync.dma_start(t[:], eT[c * 128:(c + 1) * 128, :])
                eT_tiles.append(t)
            fce_tiles = []
            for c in range(8):
                t = pfce.tile([128, H], xdt, name=f"fce{c}")
                nc.sync.dma_start(t[:], wfce[c * 128:(c + 1) * 128, :])
                fce_tiles.append(t)

            score_sb = cst.tile([128, 16], f32)
            nc.vector.memset(score_sb[:], 0.0)
            for n in range(4):           # s groups of 512
                for m in range(8):       # h_out chunks
                    px = psA.tile([128, 512], f32, name="px")
                    for c in range(8):   # h_in chunks
                        nc.tensor.matmul(
                            px[:],
                            fce_tiles[c][:, m * 128:(m + 1) * 128],
                            eT_tiles[c][:, n * 512:(n + 1) * 512],
                            start=(c == 0),
                            stop=(c == 7),
                        )
                    xT_sb = pxT.tile([128, 512], f32, name="xT_sb")
                    nc.scalar.activation(
                        xT_sb[:], px[:], AF.Tanh, bias=q_sb[:, m:m + 1]
                    )
                    scr_s = psS.tile([128, 4], f32, name="scr_s", tag="scr")
                    for mm in range(4):  # scores partials
                        nc.tensor.matmul(
                            scr_s[:, mm:mm + 1],
                            xT_sb[:, mm * 128:(mm + 1) * 128],
                            wat_sb[:, m:m + 1],
                            start=True,
                            stop=True,
                        )
                    nc.vector.tensor_add(
                        score_sb[:, n * 4:(n + 1) * 4],
                        score_sb[:, n * 4:(n + 1) * 4],
                        scr_s[:],
                    )

            # ============ softmax pieces (no max-sub; |scores|<~20) ============
            p_sb = cst.tile([128, 16], f32)       # exp(scores), unnormalized
            se_sb = cst.tile([128, 1], f32)       # per-partition sums
            nc.scalar.activation(p_sb[:], score_sb[:], AF.Exp, accum_out=se_sb[:])
            ptot = psB.tile([1, 1], f32, name="ptot", tag="psmall")
            nc.tensor.matmul(ptot[:], se_sb[:], ones_col[:], start=True, stop=True)
            tot_sb = cst.tile([1, 8], f32)
            nc.vector.memset(tot_sb[:], 0.0)
            nc.vector.tensor_copy(tot_sb[:, 0:1], ptot[:])

            # ============ context partials: ctx[h] = sum_s p[s] E[s,h] ============
            ctx_sb = cst.tile([128, 8], f32)
            nc.vector.memset(ctx_sb[:], 0.0)
            for sc in range(16):
                e_sb = pe_pool.tile([128, H], f32, name="e_sb")
                nc.sync.dma_start(e_sb[:], e_[sc * 128:(sc + 1) * 128, :])
                scr_c = psS.tile([128, 8], f32, name="scr_c", tag="scr")
                for ch in range(8):
                    nc.tensor.matmul(
                        scr_c[:, ch:ch + 1],
                        e_sb[:, ch * 128:(ch + 1) * 128],
                        p_sb[:, sc:sc + 1],
                        start=True,
                        stop=True,
                    )
                nc.vector.tensor_add(ctx_sb[:], ctx_sb[:], scr_c[:])

            # ============ AllReduce [ctx partials | sum_exp | pad] ============
            ar_in = dram.tile([1032], f32, name="ar_in")
            ar_out = dram.tile([1032], f32, name="ar_out", addr_space="Shared")
            nc.sync.dma_start(ar_in[0:1024].rearrange("(p c) -> p c", c=8), ctx_sb[:])
            nc.sync.dma_start(ar_in[1024:1032].rearrange("(o c) -> o c", o=1), tot_sb[:])
            nc.gpsimd.collective_compute(
                "AllReduce", mybir.AluOpType.add,
                replica_groups=[list(range(NCORES))],
                ins=[ar_in.opt()], outs=[ar_out.opt()],
            )

            # ============ gates: emb part (independent of AllReduce) ============
            g_acc = cst.tile([128, 4], f32)
            nc.vector.tensor_copy(g_acc[:], biasg_sb[:])
            for c in range(KEMB):
                wemb_sb = pwg.tile([128, 512], f32, name="wemb_sb")
                nc.sync.dma_start(wemb_sb[:], wemb[c * 128:(c + 1) * 128, :])
                scr_g = psS.tile([128, 4], f32, name="scr_g", tag="scr")
                for m in range(4):
                    nc.tensor.matmul(
                        scr_g[:, m:m + 1],
                        wemb_sb[:, m * 128:(m + 1) * 128],
                        emb_sb[:, c:c + 1],
                        start=True,
                        stop=True,
                    )
                nc.vector.tensor_add(g_acc[:], g_acc[:], scr_g[:])
            # gates: W_hh @ h0 part (also independent)
            for c in range(8):
                whh_sb = pwg.tile([128, 512], f32, name="whh_sb")
                nc.sync.dma_start(whh_sb[:], whh[c * 128:(c + 1) * 128, :])
                scr_g = psS.tile([128, 4], f32, name="scr_g", tag="scr")
                for m in range(4):
                    nc.tensor.matmul(
                        scr_g[:, m:m + 1],
                        whh_sb[:, m * 128:(m + 1) * 128],
                        h0_sb[:, c:c + 1],
                        start=True,
                        stop=True,
                    )
                nc.vector.tensor_add(g_acc[:], g_acc[:], scr_g[:])

            # ============ unpack AllReduce, normalize ============
            ctxg_sb = cst.tile([128, 8], f32)
            sumg_sb = cst.tile([1, 1], f32)
            nc.sync.dma_start(ctxg_sb[:], ar_out[0:1024].rearrange("(p c) -> p c", c=8))
            nc.sync.dma_start(sumg_sb[:], ar_out[1024:1025].rearrange("(o c) -> o c", o=1))
            recip_sb = cst.tile([1, 1], f32)
            nc.vector.reciprocal(recip_sb[:], sumg_sb[:])
            prb = psB.tile([128, 1], f32, name="prb", tag="psmall")
            nc.tensor.matmul(prb[:], ones_row[:], recip_sb[:], start=True, stop=True)
            rb_sb = cst.tile([128, 1], f32)
            nc.vector.tensor_copy(rb_sb[:], prb[:])
            ctxn_sb = cst.tile([128, 8], f32)
            nc.vector.tensor_scalar_mul(ctxn_sb[:], ctxg_sb[:], rb_sb[:])

            # normalized attention weights out
            pn_sb = cst.tile([128, 16], f32)
            nc.vector.tensor_scalar_mul(pn_sb[:], p_sb[:], rb_sb[:])
            nc.sync.dma_start(attn_o[:].rearrange("(p c) -> p c", c=16), pn_sb[:])

            # ============ gates: ctx part (after AllReduce) ============
            for c in range(8):
                wctx_sb = pwg.tile([128, 512], f32, name="wctx_sb")
                nc.sync.dma_start(wctx_sb[:], wctx[c * 128:(c + 1) * 128, :])
                scr_g = psS.tile([128, 4], f32, name="scr_g", tag="scr")
                for m in range(4):
                    nc.tensor.matmul(
                        scr_g[:, m:m + 1],
                        wctx_sb[:, m * 128:(m + 1) * 128],
                        ctxn_sb[:, c:c + 1],
                        start=True,
                        stop=True,
                    )
                nc.vector.tensor_add(g_acc[:], g_acc[:], scr_g[:])

            # ============ LSTM cell on the 128-hidden slice ============
            gates_sb = g_acc
            act_sb = cst.tile([128, 4], f32)
            nc.scalar.activation(act_sb[:, 0:1], gates_sb[:, 0:1], AF.Sigmoid)
            nc.scalar.activation(act_sb[:, 1:2], gates_sb[:, 1:2], AF.Sigmoid)
            nc.scalar.activation(act_sb[:, 2:3], gates_sb[:, 2:3], AF.Tanh)
            nc.scalar.activation(act_sb[:, 3:4], gates_sb[:, 3:4], AF.Sigmoid)
            t1_sb = cst.tile([128, 1], f32)
            t2_sb = cst.tile([128, 1], f32)
            cnew_sb = cst.tile([128, 1], f32)
            nc.vector.tensor_mul(t1_sb[:], act_sb[:, 1:2], c0_sb[:])
            nc.vector.tensor_mul(t2_sb[:], act_sb[:, 0:1], act_sb[:, 2:3])
            nc.vector.tensor_add(cnew_sb[:], t1_sb[:], t2_sb[:])
            tanhc_sb = cst.tile([128, 1], f32)
            nc.scalar.activation(tanhc_sb[:], cnew_sb[:], AF.Tanh)
            hnew_sb = cst.tile([128, 1], f32)
            nc.vector.tensor_mul(hnew_sb[:], act_sb[:, 3:4], tanhc_sb[:])
            nc.sync.dma_start(c_o[:].rearrange("(p o) -> p o", o=1), cnew_sb[:])
            nc.sync.dma_start(h_o[:].rearrange("(p o) -> p o", o=1), hnew_sb[:])

            # ============ AllGather h_new ============
            ag_in = dram.tile([HG], f32, name="ag_in")
            ag_out = dram.tile([H], f32, name="ag_out", addr_space="Shared")
            nc.sync.dma_start(ag_in[:].rearrange("(p o) -> p o", o=1), hnew_sb[:])
            nc.gpsimd.collective_compute(
                "AllGather", mybir.AluOpType.bypass,
                replica_groups=[list(range(NCORES))],
                ins=[ag_in.opt()], outs=[ag_out.opt()],
            )
            hn_sb = cst.tile([128, 8], f32)
            nc.sync.dma_start(hn_sb[:], ag_out[:].rearrange("(c p) -> p c", p=128))

            # ============ classifier ============
            cls_sb = cst.tile([CLS_M, CLS_NM], f32)
            nc.vector.tensor_copy(cls_sb[:], bcls_sb[:])
            QW = CLS // 4            # 1000-wide weight tiles
            QM = CLS_NM // 4         # 8 output chunks per tile
            for c in range(8):
                for part in range(4):
                    wcls_sb = pwcls.tile([128, QW], f32, name="wcls_sb")
                    nc.sync.dma_start(
                        wcls_sb[:],
                        wcls[c * 128:(c + 1) * 128, part * QW:(part + 1) * QW],
                    )
                    scr_cl = psS.tile([CLS_M, QM], f32, name="scr_cl", tag="scr")
                    for mm in range(QM):
                        nc.tensor.matmul(
                            scr_cl[:, mm:mm + 1],
                            wcls_sb[:, mm * CLS_M:(mm + 1) * CLS_M],
                            hn_sb[:, c:c + 1],
                            start=True,
                            stop=True,
                        )
                    nc.vector.tensor_add(
                        cls_sb[:, part * QM:(part + 1) * QM],
                        cls_sb[:, part * QM:(part + 1) * QM],
                        scr_cl[:],
                    )
            nc.sync.dma_start(cls_o[:].rearrange("(p c) -> p c", c=CLS_NM), cls_sb[:])

            if DEBUG_OUTS:
                dbg_q = nc.dram_tensor("dbg_q", [128, 8], f32, kind="ExternalOutput")
                dbg_p = nc.dram_tensor("dbg_p", [128, 16], f32, kind="ExternalOutput")
                dbg_se = nc.dram_tensor("dbg_se", [1, 8], f32, kind="ExternalOutput")
                dbg_ctx = nc.dram_tensor("dbg_ctx", [128, 8], f32, kind="ExternalOutput")
                dbg_ar = nc.dram_tensor("dbg_ar", [1032], f32, kind="ExternalOutput")
                dbg_g = nc.dram_tensor("dbg_g", [128, 4], f32, kind="ExternalOutput")
                dbg_hn = nc.dram_tensor("dbg_hn", [128, 8], f32, kind="ExternalOutput")
                nc.sync.dma_start(dbg_q[:], q_sb[:])
                nc.sync.dma_start(dbg_p[:], p_sb[:])
                nc.sync.dma_start(dbg_se[:], tot_sb[:])
                nc.sync.dma_start(dbg_ctx[:], ctx_sb[:])
                nc.sync.dma_start(dbg_ar[:], ar_out[:])
                nc.sync.dma_start(dbg_g[:], gates_sb[:])
                nc.sync.dma_start(dbg_hn[:], hn_sb[:])

    _legalize_single_wait(nc)
    return nc


def _prep_in_maps(inputs):
    """Slice / transpose the full inputs into the 8 per-core input maps."""
    f = np.float32
    E = np.asarray(inputs["encoder_outputs"], f)            # [S, H]
    ET = np.ascontiguousarray(E.T)                          # [H, S]
    h0 = np.asarray(inputs["h"], f).reshape(H)
    c0 = np.asarray(inputs["c"], f).reshape(H)
    emb = np.asarray(inputs["inputs"], f).reshape(O)
    wfce_t = np.ascontiguousarray(np.asarray(inputs["W_fc_encoder"], f).T)
    wfch_t = np.ascontiguousarray(np.asarray(inputs["W_fc_hidden"], f).T)
    watw = np.asarray(inputs["attn_weight"], f).reshape(H)
    W_ih = np.asarray(inputs["W_ih"], f)                    # [4H, O+H]
    W_hh = np.asarray(inputs["W_hh"], f)                    # [4H, H]
    bsum = (np.asarray(inputs["b_ih"], f) + np.asarray(inputs["b_hh"], f))  # [4H]
    W_cls = np.asarray(inputs["W_cls"], f)                  # [O, H]
    b_cls = np.asarray(inputs["b_cls"], f)                  # [O]

    wat_t = np.ascontiguousarray(watw.reshape(8, 128).T)    # [128, 8]
    h0_t = np.ascontiguousarray(h0.reshape(8, 128).T)
    emb_t = np.ascontiguousarray(emb.reshape(KEMB, 128).T)  # [128, 250]

    in_maps = []
    for k in range(NCORES):
        rows = np.concatenate(
            [np.arange(g * H + k * HG, g * H + (k + 1) * HG) for g in range(4)]
        )
        Wk = W_ih[rows]                                     # [512, O+H]
        in_maps.append({
            "eT": np.ascontiguousarray(ET[:, k * SC:(k + 1) * SC]),
            "e": np.ascontiguousarray(E[k * SC:(k + 1) * SC, :]),
            "wfce": wfce_t,
            "wfch": wfch_t,
            "wat": wat_t,
            "h0t": h0_t,
            "c0c": np.ascontiguousarray(c0[k * HG:(k + 1) * HG].reshape(HG, 1)),
            "embt": emb_t,
            "wemb": np.ascontiguousarray(Wk[:, :O].T),      # [O, 512]
            "wctx": np.ascontiguousarray(Wk[:, O:].T),      # [H, 512]
            "whh": np.ascontiguousarray(W_hh[rows].T),      # [H, 512]
            "biasg": np.ascontiguousarray(bsum[rows].reshape(4, HG).T),
            "wcls": np.ascontiguousarray(W_cls[k * CLS:(k + 1) * CLS, :].T),
            "bcls": np.ascontiguousarray(
                b_cls[k * CLS:(k + 1) * CLS].reshape(CLS_NM, CLS_M).T
            ),
        })
    return in_maps


def _assemble(results):
    f = np.float32
    out = np.concatenate(
        [r["cls_o"].reshape(CLS_M, CLS_NM).T.ravel() for r in results]
    ).astype(f).reshape(1, O)
    h_new = np.concatenate([r["h_o"] for r in results]).astype(f).reshape(1, 1, H)
    c_new = np.concatenate([r["c_o"] for r in results]).astype(f).reshape(1, 1, H)
    attn = np.concatenate(
        [r["attn_o"].reshape(128, 16).T.ravel() for r in results]
    ).astype(f).reshape(1, S)
    return out, (h_new, c_new), attn


def get_nc():
    if "nc" not in _CACHE:
        _CACHE["nc"] = _build()
    return _CACHE["nc"]


def kernel(**inputs):
    from concourse.bass_utils import run_bass_kernel_spmd

    nc = get_nc()
    in_maps = _prep_in_maps(inputs)
    res = run_bass_kernel_spmd(nc, in_maps, core_ids=list(range(NCORES)))
    return _assemble(res.results)
